# revision 8
# baseline (speedup 1.0000x reference)
"""CrossAttentionFusion Trainium2 kernel (nn_CrossAttentionFusion__45561013076033).

Full inputs -> full output. Sharding: 8 cores, core c handles batch b=c//2,
query-half h=c%2 (2048 of 4096 queries). Each core holds the full [256,4096]
cnn feature map of its batch (keys), its query-half of the transformer
features, and replicated weights.

Key restructurings vs the naive dataflow:
  * out = Wf1 @ x_trf + Wf2 @ attended + bf'.  Fold Wf2 into the value
    projection: U = (Wf2 @ Wv) @ x_cnn, so attention directly produces
    conv-ready channels; bv's contribution is constant (softmax rows sum
    to 1) and lands in bf' = bf + Wf2 @ bv.
  * Scores are computed pre-transposed, S^T[k, q] = K_kt^T Q, as ONE fp8
    DoubleRow matmul per 128-key tile (256-deep contraction in a single
    pass; Q/K are written in fp8e4m3 straight out of the projections; the
    1/sqrt(dim) scale is applied by the exp activation's free affine).
  * P^T = exp(S^T) is stored in fp8, and the PV matmul runs fp8 DoubleRow
    too, with U^T chunks stationary: psE[e, q] = sum_k U^T[k,e] P^T[k,q].
    Output lands directly in the conv layout -- no transposes anywhere.
  * Softmax row-sums R[q] are one fp8 DoubleRow matmul per key-tile pair
    against a constant 16.0-vector; 1/R is broadcast to 128 partitions by
    a K=1 matmul with a bf16 ones-column as the stationary operand (the
    16x fp8-range scaling of U and of the sum vector cancels in A/R).
  * The vector engine normalizes and combines: out = psE*(1/16R) + (Wf1
    x_trf + bf') with the Wf1 part produced by PE+ACT in parallel.
"""

import numpy as np

B, C, H, W = 4, 256, 64, 64
N = H * W            # 4096 tokens
NCORES = 8
QH = N // 2          # 2048 queries per core
CT = C // 128        # 2 channel tiles
KC = N // 512        # 8 key chunks of 512
NSB = QH // 512      # 4 superblocks per core
NKT = N // 128       # 32 key tiles

_CACHE = {}


def _build():
    import concourse.bass as bass
    import concourse.mybir as mybir
    import concourse.tile as tile
    from concourse import bacc

    f32 = mybir.dt.float32
    f32r = mybir.dt.float32r
    bf16 = mybir.dt.bfloat16
    f8 = mybir.dt.float8e4
    AF = mybir.ActivationFunctionType
    DR = mybir.MatmulPerfMode.DoubleRow

    nc = bacc.Bacc("TRN2", target_bir_lowering=False, debug=True)

    XQ = nc.dram_tensor("xq", [C, QH], f32, kind="ExternalInput")
    XC = nc.dram_tensor("xc", [C, N], f32, kind="ExternalInput")
    WQT = nc.dram_tensor("wqt", [C, C], f32, kind="ExternalInput")
    WKT = nc.dram_tensor("wkt", [C, C], f32, kind="ExternalInput")
    WUT = nc.dram_tensor("wut", [C, C], f32, kind="ExternalInput")
    WF1 = nc.dram_tensor("wf1", [C, C], f32, kind="ExternalInput")
    BQ = nc.dram_tensor("bq", [C], f32, kind="ExternalInput")
    BK = nc.dram_tensor("bk", [C], f32, kind="ExternalInput")
    BF = nc.dram_tensor("bf", [C], f32, kind="ExternalInput")
    OUT = nc.dram_tensor("out", [C, QH], f32, kind="ExternalOutput")

    xq_d = XQ.ap().bitcast(f32r).rearrange("(t p) n -> p t n", p=128)
    xc_d = XC.ap().bitcast(f32r).rearrange("(t p) n -> p t n", p=128)
    wq_d = WQT.ap().bitcast(f32r).rearrange("(t p) d -> p t d", p=128)
    wk_d = WKT.ap().bitcast(f32r).rearrange("(t p) d -> p t d", p=128)
    wu_d = WUT.ap().bitcast(f32r).rearrange("(t p) d -> p t d", p=128)
    wf_d = WF1.ap().bitcast(f32r).rearrange("(t p) d -> p t d", p=128)
    out_d = OUT.ap().rearrange("(t p) n -> p t n", p=128)

    with tile.TileContext(nc) as tc:
        with tc.tile_pool(name="persist", bufs=1) as per, \
             tc.tile_pool(name="pt", bufs=2) as ptp, \
             tc.tile_pool(name="cb", bufs=4) as cbp, \
             tc.tile_pool(name="rbp", bufs=2) as rbp, \
             tc.tile_pool(name="outp", bufs=2) as outp, \
             tc.tile_pool(name="mm", bufs=2, space="PSUM") as mmp, \
             tc.tile_pool(name="pe", bufs=1, space="PSUM") as pep, \
             tc.tile_pool(name="pr", bufs=1, space="PSUM") as prp, \
             tc.tile_pool(name="po", bufs=1, space="PSUM") as pop:

            # ---- persistent tiles ----
            xq_sb = per.tile([128, CT, QH], f32r)
            xc_sb = per.tile([128, CT, N], f32r)
            wq_sb = per.tile([128, CT, C], f32r)
            wk_sb = per.tile([128, CT, C], f32r)
            wu_sb = per.tile([128, CT, C], f32r)
            wf_sb = per.tile([128, CT, C], f32r)
            bq_sb = per.tile([128, CT], f32)
            bk_sb = per.tile([128, CT], f32)
            bf_sb = per.tile([128, CT], f32)
            q8_sb = per.tile([128, CT, QH], f8)
            k8_sb = per.tile([128, CT, N], f8)
            xc8_sb = per.tile([128, CT, N], f8)
            wu8_sb = per.tile([128, CT, C], f8)
            ut8_sb = per.tile([128, NKT, C], f8)
            ones8 = per.tile([128, CT, 16], f8)
            onesb = per.tile([1, 128], bf16)

            nc.sync.dma_start(bq_sb[:], BQ.ap().rearrange("(t p) -> p t", p=128))
            nc.sync.dma_start(bk_sb[:], BK.ap().rearrange("(t p) -> p t", p=128))
            nc.sync.dma_start(bf_sb[:], BF.ap().rearrange("(t p) -> p t", p=128))
            nc.sync.dma_start(wq_sb[:], wq_d)
            nc.gpsimd.memset(ones8[:], 16.0)
            nc.gpsimd.memset(onesb[:], 1.0)

            # input DMAs: xq/xc interleaved 512-column chunks so both Q and
            # K projections start early; remaining weights between chunks
            for i in range(KC):
                if i < QH // 512:
                    for ct in range(CT):
                        s = slice(i * 512, (i + 1) * 512)
                        nc.sync.dma_start(xq_sb[:, ct, s], xq_d[:, ct, s])
                if i == 0:
                    nc.sync.dma_start(wk_sb[:], wk_d)
                elif i == 1:
                    nc.sync.dma_start(wu_sb[:], wu_d)
                elif i == 2:
                    nc.sync.dma_start(wf_sb[:], wf_d)
                for ct in range(CT):
                    s = slice(i * 512, (i + 1) * 512)
                    nc.sync.dma_start(xc_sb[:, ct, s], xc_d[:, ct, s])

            nc.vector.tensor_scalar_mul(wu8_sb[:], wu_sb[:], 16.0)

            # ---- Q projection -> fp8 (scores keep full scale; the 1/16
            # softmax scale is applied inside the exp activation) ----
            for qc in range(QH // 512):
                s = slice(qc * 512, (qc + 1) * 512)
                for dt in range(CT):
                    ps = mmp.tile([128, 2, 512], f32, tag="mm")
                    for ct in range(CT):
                        nc.tensor.matmul(
                            ps[:, 0], wq_sb[:, ct, dt * 128:(dt + 1) * 128],
                            xq_sb[:, ct, s],
                            start=(ct == 0), stop=(ct == CT - 1))
                    nc.scalar.activation(q8_sb[:, dt, s], ps[:, 0],
                                         AF.Identity, bias=bq_sb[:, dt:dt + 1])

            # ---- K projection -> fp8 ----
            for kc in range(KC):
                s = slice(kc * 512, (kc + 1) * 512)
                for dt in range(CT):
                    ps = mmp.tile([128, 2, 512], f32, tag="mm")
                    for ct in range(CT):
                        nc.tensor.matmul(
                            ps[:, 0], wk_sb[:, ct, dt * 128:(dt + 1) * 128],
                            xc_sb[:, ct, s],
                            start=(ct == 0), stop=(ct == CT - 1))
                    nc.scalar.activation(k8_sb[:, dt, s], ps[:, 0],
                                         AF.Identity, bias=bk_sb[:, dt:dt + 1])
                nc.vector.tensor_copy(xc8_sb[:, :, s], xc_sb[:, :, s])

            # ---- U^T = x_cnn^T (16 Wu)^T  [keys, 256], fp8 DoubleRow ----
            for g in range(NKT // 2):
                ps = mmp.tile([128, 2, 512], f32, tag="mm")
                for j in range(2):
                    mt = 2 * g + j
                    nc.tensor.matmul(
                        ps[:, j, :C], xc8_sb[:, :, mt * 128:(mt + 1) * 128],
                        wu8_sb[:], perf_mode=DR, start=True, stop=True)
                nc.vector.tensor_copy(ut8_sb[:, 2 * g:2 * g + 2], ps[:, :, :C])

            # ---- attention + fused conv, per 512-query superblock ----
            for sb in range(NSB):
                qs = slice(sb * 512, (sb + 1) * 512)
                pt8 = ptp.tile([128, NKT, 512], f8, tag="pt")
                # S^T = K_kt^T Q_sb (fp8 DoubleRow); P^T = exp(S^T / 16)
                for g in range(NKT // 2):
                    ps = mmp.tile([128, 2, 512], f32, tag="mm")
                    for j in range(2):
                        kt = 2 * g + j
                        nc.tensor.matmul(
                            ps[:, j], k8_sb[:, :, kt * 128:(kt + 1) * 128],
                            q8_sb[:, :, qs], perf_mode=DR,
                            start=True, stop=True)
                    nc.scalar.activation(pt8[:, 2 * g:2 * g + 2], ps[:],
                                         AF.Exp, scale=0.0625)

                # conv part 1: o1 = Wf1 x_trf + bf' (PE + ACT)
                o1 = []
                for et in range(CT):
                    po = pop.tile([128, 512], f32, tag="po")
                    for ct in range(CT):
                        nc.tensor.matmul(
                            po[:], wf_sb[:, ct, et * 128:(et + 1) * 128],
                            xq_sb[:, ct, qs],
                            start=(ct == 0), stop=(ct == CT - 1))
                    o1_sb = outp.tile([128, 512], f32, tag=f"o1_{et}",
                                      name=f"o1_{et}")
                    nc.scalar.activation(o1_sb[:], po[:],
                                         AF.Identity, bias=bf_sb[:, et:et + 1])
                    o1.append(o1_sb)

                # row sums: psr[0] = 16 R[q], then 1/(16R) broadcast to all
                # partitions via a K=1 matmul
                psr = prp.tile([128, 512], f32, tag="pr")
                for g in range(NKT // 2):
                    nc.tensor.matmul(
                        psr[0:1], ones8[:, :, 0:1], pt8[:, 2 * g:2 * g + 2, :],
                        perf_mode=DR,
                        start=(g == 0), stop=(g == NKT // 2 - 1))
                rr = cbp.tile([1, 512], f32, tag="rr")
                nc.vector.reciprocal(rr[:], psr[0:1])
                rrb = cbp.tile([1, 512], bf16, tag="rrb")
                nc.vector.tensor_copy(rrb[:], rr[:])

                # PV: psE[e, q] = sum_k U^T[k, e] P^T[k, q]  (fp8 DoubleRow)
                pse = []
                for et in range(CT):
                    ps = pep.tile([128, 512], f32, tag=f"pe{et}",
                                  name=f"pse{et}")
                    for g in range(NKT // 2):
                        nc.tensor.matmul(
                            ps[:],
                            ut8_sb[:, 2 * g:2 * g + 2, et * 128:(et + 1) * 128],
                            pt8[:, 2 * g:2 * g + 2, :], perf_mode=DR,
                            start=(g == 0), stop=(g == NKT // 2 - 1))
                    pse.append(ps)

                # broadcast 1/(16R) into psr (overwrites the row sums)
                nc.tensor.matmul(psr[:], onesb[:], rrb[:],
                                 start=True, stop=True, skip_group_check=True)
                rb_sb = rbp.tile([128, 512], f32, tag="rb")
                nc.vector.tensor_copy(rb_sb[:], psr[:])

                # combine: out = psE * (1/16R) + o1, on DVE
                for et in range(CT):
                    tmp = cbp.tile([128, 512], f32, tag=f"tmp{et}",
                                   name=f"tmp{et}")
                    nc.vector.tensor_mul(tmp[:], pse[et][:], rb_sb[:])
                    o_sb = outp.tile([128, 512], f32, tag="o")
                    nc.vector.tensor_add(o_sb[:], tmp[:], o1[et][:])
                    nc.sync.dma_start(out_d[:, et, qs], o_sb[:])
    nc.finalize()
    return nc


def _get_nc():
    if "nc" not in _CACHE:
        _CACHE["nc"] = _build()
    return _CACHE["nc"]


def _in_maps(transformer_features, cnn_features, Wq, bq, Wk, bk, Wv, bv, Wf, bf):
    xt = np.ascontiguousarray(np.asarray(transformer_features, np.float32)
                              .reshape(B, C, N))
    xc = np.ascontiguousarray(np.asarray(cnn_features, np.float32)
                              .reshape(B, C, N))
    Wq = np.asarray(Wq, np.float32)
    Wk = np.asarray(Wk, np.float32)
    Wv = np.asarray(Wv, np.float32)
    Wf = np.asarray(Wf, np.float32)
    bq = np.asarray(bq, np.float32)
    bk = np.asarray(bk, np.float32)
    bv = np.asarray(bv, np.float32)
    bf = np.asarray(bf, np.float32)

    Wf1, Wf2 = Wf[:, :C], Wf[:, C:]
    wqt = np.ascontiguousarray(Wq.T)
    wkt = np.ascontiguousarray(Wk.T)
    wut = np.ascontiguousarray((Wf2 @ Wv).T)
    wf1 = np.ascontiguousarray(Wf1.T)
    bf2 = bf + Wf2 @ bv

    maps = []
    for c in range(NCORES):
        b, h = divmod(c, 2)
        maps.append(dict(
            xq=np.ascontiguousarray(xt[b][:, h * QH:(h + 1) * QH]),
            xc=xc[b],
            wqt=wqt, wkt=wkt, wut=wut, wf1=wf1,
            bq=bq, bk=bk, bf=bf2,
        ))
    return maps


def _run(inputs, trace=False):
    from concourse.bass_utils import run_bass_kernel_spmd
    nc = _get_nc()
    maps = _in_maps(**inputs)
    return run_bass_kernel_spmd(nc, maps, list(range(NCORES)), trace=trace)


def kernel(**inputs) -> np.ndarray:
    res = _run(inputs).results
    out = np.empty((B, C, N), np.float32)
    for c in range(NCORES):
        b, h = divmod(c, 2)
        out[b][:, h * QH:(h + 1) * QH] = res[c]["out"]
    return out.reshape(B, C, H, W)


# revision 9
# speedup vs baseline: 1.1026x; 1.1026x over previous
"""CrossAttentionFusion Trainium2 kernel (nn_CrossAttentionFusion__45561013076033).

Full inputs -> full output. Sharding: 8 cores, core c handles batch b=c//2,
query-half h=c%2 (2048 of 4096 queries). Each core holds the full [256,4096]
cnn feature map of its batch (keys), its query-half of the transformer
features, and replicated weights.

Key restructurings vs the naive dataflow:
  * out = Wf1 @ x_trf + Wf2 @ attended + bf'.  Fold Wf2 into the value
    projection: U = (Wf2 @ Wv) @ x_cnn, so attention directly produces
    conv-ready channels; bv's contribution is constant (softmax rows sum
    to 1) and lands in bf' = bf + Wf2 @ bv.
  * Q/K projections and scores all run as fp8e4m3 DoubleRow matmuls
    (256-deep contraction in one pass).  Host ships x_trf/x_cnn in fp8
    and the projection weights in fp8 scaled by 8 (fp8-friendly range);
    the combined 1/(16*64) score scale is applied by the exp free affine.
  * Scores are computed pre-transposed, S^T[k, q] = K_kt^T Q, so the P^T
    needed by the PV matmul comes straight out of exp() -- no transpose
    of the [N, N] attention matrix.
  * Softmax row-sums come free as a 257th column of the PV matmul by
    augmenting U^T with a constant-16 column: [16A | 16R] = P [16U | 16]
    (16x keeps (Wf2 Wv) entries inside fp8 range; cancels in A/R).
  * Normalization (1/16R per query) is applied by the vector engine while
    moving the PV result PSUM->SBUF (per-partition scalar multiply,
    queries on partitions there).
  * The [q, e] -> [e, q] layout fix-up is a matmul with a 128x128 identity
    as the moving operand, accumulated directly into the Wf1 PSUM group
    (Wf1 runs in bf16: separate, pipelined weight loads).
"""

import numpy as np

B, C, H, W = 4, 256, 64, 64
N = H * W            # 4096 tokens
NCORES = 8
QH = N // 2          # 2048 queries per core
CT = C // 128        # 2 channel tiles
KC = N // 512        # 8 key chunks of 512
NSB = QH // 512      # 4 superblocks per core
NKT = N // 128       # 32 key tiles

_CACHE = {}


def _build():
    import concourse.bass as bass
    import concourse.mybir as mybir
    import concourse.tile as tile
    from concourse import bacc
    from concourse.masks import make_identity

    f32 = mybir.dt.float32
    bf16 = mybir.dt.bfloat16
    f16 = mybir.dt.float16
    f8 = mybir.dt.float8e4
    AF = mybir.ActivationFunctionType
    DR = mybir.MatmulPerfMode.DoubleRow

    nc = bacc.Bacc("TRN2", target_bir_lowering=False, debug=True)

    XQ8 = nc.dram_tensor("xq8", [C, QH], f8, kind="ExternalInput")
    XC8 = nc.dram_tensor("xc8", [C, N], f8, kind="ExternalInput")
    XQB = nc.dram_tensor("xqb", [C, QH], bf16, kind="ExternalInput")
    WQ8 = nc.dram_tensor("wq8", [C, C], f8, kind="ExternalInput")
    WK8 = nc.dram_tensor("wk8", [C, C], f8, kind="ExternalInput")
    WU8 = nc.dram_tensor("wu8", [C, C], f8, kind="ExternalInput")
    WF1 = nc.dram_tensor("wf1", [C, C], bf16, kind="ExternalInput")
    BQ = nc.dram_tensor("bq", [C], f32, kind="ExternalInput")
    BK = nc.dram_tensor("bk", [C], f32, kind="ExternalInput")
    BF = nc.dram_tensor("bf", [C], f32, kind="ExternalInput")
    OUT = nc.dram_tensor("out", [C, QH], f32, kind="ExternalOutput")

    xq8_d = XQ8.ap().rearrange("(t p) n -> p t n", p=128)
    xc8_d = XC8.ap().rearrange("(t p) n -> p t n", p=128)
    xqb_d = XQB.ap().rearrange("(t p) n -> p t n", p=128)
    wq_d = WQ8.ap().rearrange("(t p) d -> p t d", p=128)
    wk_d = WK8.ap().rearrange("(t p) d -> p t d", p=128)
    wu_d = WU8.ap().rearrange("(t p) d -> p t d", p=128)
    wf_d = WF1.ap().rearrange("(t p) d -> p t d", p=128)
    out_d = OUT.ap().rearrange("(t p) n -> p t n", p=128)

    with tile.TileContext(nc) as tc:
        with tc.tile_pool(name="persist", bufs=1) as per, \
             tc.tile_pool(name="pt", bufs=2) as ptp, \
             tc.tile_pool(name="cb", bufs=4) as cbp, \
             tc.tile_pool(name="outp", bufs=2) as outp, \
             tc.tile_pool(name="mm", bufs=2, space="PSUM") as mmp, \
             tc.tile_pool(name="pv", bufs=2, space="PSUM") as pvp, \
             tc.tile_pool(name="po", bufs=1, space="PSUM") as pop:

            # ---- persistent tiles ----
            xq8_sb = per.tile([128, CT, QH], f8)
            xc8_sb = per.tile([128, CT, N], f8)
            xqb_sb = per.tile([128, CT, QH], bf16)
            wq_sb = per.tile([128, CT, C], f8)
            wk_sb = per.tile([128, CT, C], f8)
            wu_sb = per.tile([128, CT, C], f8)
            wf_sb = per.tile([128, CT, C], bf16)
            bq_sb = per.tile([128, CT], f32)
            bk_sb = per.tile([128, CT], f32)
            bf_sb = per.tile([128, CT], f32)
            q8_sb = per.tile([128, CT, QH], f8)
            k8_sb = per.tile([128, CT, N], f8)
            ut_sb = per.tile([128, NKT, C + 1], f16)
            ident = per.tile([128, 128], f16)

            nc.sync.dma_start(bq_sb[:], BQ.ap().rearrange("(t p) -> p t", p=128))
            nc.sync.dma_start(bk_sb[:], BK.ap().rearrange("(t p) -> p t", p=128))
            nc.sync.dma_start(bf_sb[:], BF.ap().rearrange("(t p) -> p t", p=128))
            nc.sync.dma_start(wq_sb[:], wq_d)
            nc.sync.dma_start(wk_sb[:], wk_d)
            nc.sync.dma_start(wu_sb[:], wu_d)
            nc.sync.dma_start(wf_sb[:], wf_d)
            make_identity(nc, ident[:])
            nc.gpsimd.memset(ut_sb[:, :, C:C + 1], 16.0)

            # input DMAs: fp8 features first (everything depends on them),
            # bf16 x_trf (Wf1 path) afterwards
            for i in range(KC):
                if i < QH // 512:
                    for ct in range(CT):
                        s = slice(i * 512, (i + 1) * 512)
                        nc.sync.dma_start(xq8_sb[:, ct, s], xq8_d[:, ct, s])
                for ct in range(CT):
                    s = slice(i * 512, (i + 1) * 512)
                    nc.sync.dma_start(xc8_sb[:, ct, s], xc8_d[:, ct, s])
            for i in range(QH // 512):
                for ct in range(CT):
                    s = slice(i * 512, (i + 1) * 512)
                    nc.sync.dma_start(xqb_sb[:, ct, s], xqb_d[:, ct, s])

            # ---- Q projection (fp8 DoubleRow) -> fp8, values are 8q ----
            for qc in range(QH // 512):
                s = slice(qc * 512, (qc + 1) * 512)
                for dt in range(CT):
                    ps = mmp.tile([128, 2, 512], f32, tag="mm")
                    nc.tensor.matmul(
                        ps[:, 0], wq_sb[:, :, dt * 128:(dt + 1) * 128],
                        xq8_sb[:, :, s], perf_mode=DR, start=True, stop=True)
                    nc.scalar.activation(q8_sb[:, dt, s], ps[:, 0],
                                         AF.Identity, bias=bq_sb[:, dt:dt + 1])

            # ---- K projection (fp8 DoubleRow) -> fp8, values are 8k ----
            for kc in range(KC):
                s = slice(kc * 512, (kc + 1) * 512)
                for dt in range(CT):
                    ps = mmp.tile([128, 2, 512], f32, tag="mm")
                    nc.tensor.matmul(
                        ps[:, 0], wk_sb[:, :, dt * 128:(dt + 1) * 128],
                        xc8_sb[:, :, s], perf_mode=DR, start=True, stop=True)
                    nc.scalar.activation(k8_sb[:, dt, s], ps[:, 0],
                                         AF.Identity, bias=bk_sb[:, dt:dt + 1])

            # ---- U^T = x_cnn^T (16 Wu)^T  [keys, 256], fp8 DoubleRow ----
            for g in range(NKT // 2):
                ps = mmp.tile([128, 2, 512], f32, tag="mm")
                for j in range(2):
                    mt = 2 * g + j
                    nc.tensor.matmul(
                        ps[:, j, :C], xc8_sb[:, :, mt * 128:(mt + 1) * 128],
                        wu_sb[:], perf_mode=DR, start=True, stop=True)
                nc.vector.tensor_copy(ut_sb[:, 2 * g:2 * g + 2, :C],
                                      ps[:, :, :C])

            # ---- attention + fused conv, per 512-query superblock ----
            # score scale: S = (8q . 8k) = 64 s ; softmax wants s/16
            escale = 1.0 / (16.0 * 64.0)
            for sb in range(NSB):
                qs = slice(sb * 512, (sb + 1) * 512)
                pt_sb = ptp.tile([128, NKT, 512], f16, tag="pt")
                # S^T = K_kt^T Q_sb (fp8 DoubleRow); P^T = exp(S^T/1024)
                for g in range(NKT // 2):
                    ps = mmp.tile([128, 2, 512], f32, tag="mm")
                    for j in range(2):
                        kt = 2 * g + j
                        nc.tensor.matmul(
                            ps[:, j], k8_sb[:, :, kt * 128:(kt + 1) * 128],
                            q8_sb[:, :, qs], perf_mode=DR,
                            start=True, stop=True)
                    nc.scalar.activation(pt_sb[:, 2 * g:2 * g + 2], ps[:],
                                         AF.Exp, scale=escale)

                # conv part 1 into psO (bf16 weights: pipelined LDW)
                pso = [pop.tile([128, 512], f32, tag=f"po{et}", name=f"pso{et}")
                       for et in range(CT)]
                for et in range(CT):
                    for ct in range(CT):
                        nc.tensor.matmul(
                            pso[et][:], wf_sb[:, ct, et * 128:(et + 1) * 128],
                            xqb_sb[:, ct, qs],
                            start=(ct == 0), stop=False)

                # PV: [16A | 16R] per 128-query block, normalize on DVE
                c_blk = []
                for qj in range(4):
                    psb = pvp.tile([128, C + 1], f32, tag="pv")
                    for kt in range(NKT):
                        nc.tensor.matmul(
                            psb[:], pt_sb[:, kt, qj * 128:(qj + 1) * 128],
                            ut_sb[:, kt],
                            start=(kt == 0), stop=(kt == NKT - 1))
                    rinv = cbp.tile([128, 1], f32, tag="rinv")
                    nc.vector.reciprocal(rinv[:], psb[:, C:C + 1])
                    c_sb = cbp.tile([128, C], f16, tag="c")
                    nc.vector.tensor_scalar_mul(c_sb[:], psb[:, :C], rinv[:])
                    c_blk.append(c_sb)

                # transpose each c block into the psO accumulation via
                # identity-matmul; last one closes the group
                for qj in range(4):
                    for et in range(CT):
                        nc.tensor.matmul(
                            pso[et][:, qj * 128:(qj + 1) * 128],
                            c_blk[qj][:, et * 128:(et + 1) * 128],
                            ident[:],
                            start=False, stop=(qj == 3),
                            skip_group_check=True)

                for et in range(CT):
                    o_sb = outp.tile([128, 512], f32, tag="o")
                    nc.scalar.activation(o_sb[:], pso[et][:],
                                         AF.Identity, bias=bf_sb[:, et:et + 1])
                    nc.sync.dma_start(out_d[:, et, qs], o_sb[:])
    nc.finalize()
    return nc


def _get_nc():
    if "nc" not in _CACHE:
        _CACHE["nc"] = _build()
    return _CACHE["nc"]


def _in_maps(transformer_features, cnn_features, Wq, bq, Wk, bk, Wv, bv, Wf, bf):
    import ml_dtypes
    f8 = ml_dtypes.float8_e4m3fn

    xt = np.ascontiguousarray(np.asarray(transformer_features, np.float32)
                              .reshape(B, C, N))
    xc = np.ascontiguousarray(np.asarray(cnn_features, np.float32)
                              .reshape(B, C, N))
    Wq = np.asarray(Wq, np.float32)
    Wk = np.asarray(Wk, np.float32)
    Wv = np.asarray(Wv, np.float32)
    Wf = np.asarray(Wf, np.float32)
    bq = np.asarray(bq, np.float32)
    bk = np.asarray(bk, np.float32)
    bv = np.asarray(bv, np.float32)
    bf = np.asarray(bf, np.float32)

    Wf1, Wf2 = Wf[:, :C], Wf[:, C:]
    wq8 = np.ascontiguousarray(8.0 * Wq.T).astype(f8)
    wk8 = np.ascontiguousarray(8.0 * Wk.T).astype(f8)
    wu8 = np.ascontiguousarray(16.0 * (Wf2 @ Wv).T).astype(f8)
    wf1 = np.ascontiguousarray(Wf1.T).astype(ml_dtypes.bfloat16)
    bf2 = bf + Wf2 @ bv
    xc8 = xc.astype(f8)

    maps = []
    for c in range(NCORES):
        b, h = divmod(c, 2)
        xq = np.ascontiguousarray(xt[b][:, h * QH:(h + 1) * QH])
        maps.append(dict(
            xq8=xq.astype(f8),
            xc8=xc8[b],
            xqb=xq.astype(ml_dtypes.bfloat16),
            wq8=wq8, wk8=wk8, wu8=wu8, wf1=wf1,
            bq=8.0 * bq, bk=8.0 * bk, bf=bf2,
        ))
    return maps


def _run(inputs, trace=False):
    from concourse.bass_utils import run_bass_kernel_spmd
    nc = _get_nc()
    maps = _in_maps(**inputs)
    return run_bass_kernel_spmd(nc, maps, list(range(NCORES)), trace=trace)


def kernel(**inputs) -> np.ndarray:
    res = _run(inputs).results
    out = np.empty((B, C, N), np.float32)
    for c in range(NCORES):
        b, h = divmod(c, 2)
        out[b][:, h * QH:(h + 1) * QH] = res[c]["out"]
    return out.reshape(B, C, H, W)
